# revision 1
# baseline (speedup 1.0000x reference)
"""Distributed Trainium2 kernel for decode-style multi-head attention.

Shape: B=8, S=16, H=32, D=64, HID=2048, PAST=4096 (T=4112 after concat).
Sharding: tensor-parallel over heads — each of 8 cores owns 4 heads:
  wq/wk/wv row-sharded (output features), wo column-sharded (input features),
  past KV naturally per-head; partial out-proj summed with chunked
  ReduceScatters (each core returns only its 16-row shard; the host
  assembles the full output from all 8 cores).

Per-core dataflow (all matmuls out = lhsT.T @ rhs, contract on partitions;
every matmul operand is kept at base partition 0 — base-64 operands fault
on this runtime; partition moves go through SBUF->SBUF DMA instead):
  - x [128,2048] transposed on PE -> xT (bf16).
  - weight shards transposed on PE -> wqT/wkT/wvT [hid,256], woT [256,2048] bf16.
  - projections: qT/kT [256,128] (head-dim major), v [128,256] (token major).
  - per (b,h) pair: KV DMA'd 512B-interleaved (partition p <- tokens 2p,2p+1
    of each 256-token group; consecutive 64-col slices are 128-token tiles),
    K tiles PE-transposed one by one -> kT [64,128] tiles (base 0), extracted
    f32->bf16 alternating ACT/DVE; scores^T = stationary kT x moving qT ->
    PSUM [128tok,16]; exp on ACT -> probsT bf16; out2^T accumulated as
    stationary [v|1] bf16 x moving probsT -> PSUM [65,16] (row 64 = denom).
  - normalize via reciprocal + gpsimd partition_broadcast + multiply into a
    base-0 staging; attnT assembled by rebase DMAs per 2-batch chunk; chunked
    out-proj -> ReduceScatter (0.25 MB in, 4 rows out per core) overlapped
    with the next chunk's compute; single output DMA at the end.
"""

import os

import numpy as np

import concourse.bass as bass
import concourse.mybir as mybir
import concourse.tile as tile
from concourse import bacc
from concourse.masks import make_identity
from concourse.bass_utils import run_bass_kernel_spmd

F32 = mybir.dt.float32
BF16 = mybir.dt.bfloat16

B, S, H, D = 8, 16, 32, 64
HID = H * D            # 2048
PAST = 4096
NCORES = 8
HLOC = H // NCORES     # 4 heads per core
SH = HLOC * D          # 256 local head dims
NTOK = B * S           # 128 query tokens
NT = PAST // 128       # 32 full KV tiles (even/odd interleave)
SCALE = 1.0 / float(np.sqrt(D))
EXP = mybir.ActivationFunctionType.Exp


def build_nc():
    skip_cc = os.environ.get("SKIP_CC", "0") == "1"
    kvb = int(os.environ.get("KVB", "3"))
    sbb = int(os.environ.get("SBB", "3"))
    main_part = int(os.environ.get("MAIN_PART", "4"))
    nc = bacc.Bacc(None, target_bir_lowering=False, debug=False, num_devices=NCORES)

    hid_e = nc.declare_dram_parameter("hidden", [NTOK, HID], F32, isOutput=False)
    wq_e = nc.declare_dram_parameter("wq", [SH, HID], F32, isOutput=False)
    wk_e = nc.declare_dram_parameter("wk", [SH, HID], F32, isOutput=False)
    wv_e = nc.declare_dram_parameter("wv", [SH, HID], F32, isOutput=False)
    wo_e = nc.declare_dram_parameter("wo", [HID, SH], F32, isOutput=False)
    pk_e = nc.declare_dram_parameter("pk", [B, HLOC, PAST, D], F32, isOutput=False)
    pv_e = nc.declare_dram_parameter("pv", [B, HLOC, PAST, D], F32, isOutput=False)
    out_e = nc.declare_dram_parameter("out", [16, HID], F32, isOutput=True)

    cc_in = nc.dram_tensor("cc_in", [NTOK, HID], F32)
    cc_out = nc.dram_tensor("cc_out", [16, HID], F32)

    with tile.TileContext(nc) as tc:
        with (
            tc.tile_pool(name="const", bufs=1) as constp,
            tc.tile_pool(name="pers", bufs=1) as pers,
            tc.tile_pool(name="wload", bufs=2) as wload,
            tc.tile_pool(name="kvload", bufs=kvb) as kvload,
            tc.tile_pool(name="ktp", bufs=sbb) as ktp,
            tc.tile_pool(name="vbfp", bufs=sbb) as vbfp,
            tc.tile_pool(name="probsp", bufs=sbb) as probsp,
            tc.tile_pool(name="finp", bufs=sbb) as finp,
            tc.tile_pool(name="psA", bufs=int(os.environ.get("PSA","3")), space="PSUM") as psA,
            tc.tile_pool(name="psB", bufs=int(os.environ.get("PSB","2")), space="PSUM") as psB,
            tc.tile_pool(name="psC", bufs=int(os.environ.get("PSC","3")), space="PSUM") as psC,
        ):
            ident = constp.tile([128, 128], F32, tag="idf")
            make_identity(nc, ident[:, :])
            ident_bf = constp.tile([128, 128], BF16, tag="idb")
            make_identity(nc, ident_bf[:, :])

            # persistent per-core tensors
            xT = pers.tile([128, 16 * 128], BF16, tag="xT")
            wqT = pers.tile([128, 16 * 256], BF16, tag="wqT")
            wkT = pers.tile([128, 16 * 256], BF16, tag="wkT")
            wvT = pers.tile([128, 16 * 256], BF16, tag="wvT")
            woT = pers.tile([128, 2 * 2048], BF16, tag="woT")
            qstage = pers.tile([128, 256], BF16, tag="qstage")
            kstage = pers.tile([128, 256], BF16, tag="kstage")
            qT2 = pers.tile([64, 4 * 128], BF16, tag="qT2")       # [d, hl*128+(b,s)]
            kTn = pers.tile([64, 4 * 128], BF16, tag="kTn")
            vn2 = pers.tile([16, 32 * 65], BF16, tag="vn2")
            attnS = pers.tile([64, 4 * 128], BF16, tag="attnS")   # normalized out2^T
            attnT = pers.tile([128, 2 * 128], BF16, tag="attnT")

            def load_kv(b, hl):
                kb = kvload.tile([128, 2048], F32, tag="kbuf")
                vb = kvload.tile([128, 2048], F32, tag="vbuf")
                NSPL = 2
                ctok = 4096 // NSPL          # tokens per split
                ccol = 2048 // NSPL          # sbuf cols per split
                gs = ctok // 1024            # 1024-token groups per split
                # partition p <- tokens {8p..8p+7} of each 1024-token group:
                # 2KB contiguous DMA runs; consecutive 64-col slices are still
                # 128-token tiles (tile t = g*8+q), and V uses the identical
                # interleave so probsT/v token slots stay consistent.
                for hv in range(NSPL):
                    nc.sync.dma_start(
                        out=kb[:, hv * ccol:(hv + 1) * ccol].rearrange(
                            "p (g eight d) -> p g eight d", g=gs, eight=8, d=64),
                        in_=pk_e[b, hl, hv * ctok:(hv + 1) * ctok, :].rearrange(
                            "(g p eight) d -> p g eight d", g=gs, p=128, eight=8
                        ),
                    )
                    nc.sync.dma_start(
                        out=vb[:, hv * ccol:(hv + 1) * ccol].rearrange(
                            "p (g eight d) -> p g eight d", g=gs, eight=8, d=64),
                        in_=pv_e[b, hl, hv * ctok:(hv + 1) * ctok, :].rearrange(
                            "(g p eight) d -> p g eight d", g=gs, p=128, eight=8
                        ),
                    )
                return kb, vb

            # prefetch the first pairs' KV ahead of the weight loads so the
            # DMA queue starts on the big stream immediately
            PREF = int(os.environ.get("PREF", "0"))
            prefetch = {}
            for jp in range(PREF):
                prefetch[jp] = load_kv(jp // HLOC, jp % HLOC)

            # ---------- x load + transpose ----------
            xsb = wload.tile([128, 2048], F32, tag="wnat")
            nc.sync.dma_start(out=xsb[:, :], in_=hid_e[:, :])
            for r4 in range(4):
                ps = psA.tile([128, 512], F32, tag="trp")
                for j in range(4):
                    r = r4 * 4 + j
                    nc.tensor.transpose(
                        ps[:, j * 128:(j + 1) * 128],
                        xsb[:, r * 128:(r + 1) * 128],
                        ident[:, :],
                    )
                nc.scalar.copy(xT[:, r4 * 512:(r4 + 1) * 512], ps[:, :])

            # ---------- wq/wk/wv transposes ----------
            for w_e, dst in ((wq_e, wqT), (wk_e, wkT), (wv_e, wvT)):
                for p in range(2):
                    wn = wload.tile([128, 2048], F32, tag="wnat")
                    nc.sync.dma_start(out=wn[:, :], in_=w_e[p * 128:(p + 1) * 128, :])
                    for r4 in range(4):
                        ps = psA.tile([128, 512], F32, tag="trp")
                        for j in range(4):
                            r = r4 * 4 + j
                            nc.tensor.transpose(
                                ps[:, j * 128:(j + 1) * 128],
                                wn[:, r * 128:(r + 1) * 128],
                                ident[:, :],
                            )
                        dview = dst[:, :].rearrange("q (r c) -> q r c", r=16, c=256)
                        nc.scalar.copy(
                            dview[:, r4 * 4:(r4 + 1) * 4, p * 128:(p + 1) * 128],
                            ps[:, :].rearrange("q (j c) -> q j c", j=4, c=128),
                        )

            # ---------- wo transposes: woT[q, kc*2048 + n] = wo[n, kc*128+q]
            for hhalf in range(2):
                wn = wload.tile([128, 2048], F32, tag="wnat")
                nc.sync.dma_start(
                    out=wn[:, :],
                    in_=wo_e[:, :].rearrange("(rr p) c -> p rr c", p=128)[
                        :, hhalf * 8:(hhalf + 1) * 8, :
                    ],
                )
                for kc in range(2):
                    for rr4 in range(2):
                        ps = psA.tile([128, 512], F32, tag="trp")
                        for j in range(4):
                            rr_rel = rr4 * 4 + j
                            nc.tensor.transpose(
                                ps[:, j * 128:(j + 1) * 128],
                                wn[:, rr_rel * 256 + kc * 128: rr_rel * 256 + (kc + 1) * 128],
                                ident[:, :],
                            )
                        base = kc * 2048 + (hhalf * 8 + rr4 * 4) * 128
                        nc.scalar.copy(woT[:, base: base + 512], ps[:, :])

            # ---------- projections ----------
            for wTsrc, stg, scl in ((wqT, qstage, SCALE), (wkT, kstage, 1.0)):
                for p in range(2):
                    ps = psB.tile([128, 256], F32, tag="sc")
                    for r in range(16):
                        nc.tensor.matmul(
                            ps[:, 0:128],
                            lhsT=wTsrc[:, r * 256 + p * 128: r * 256 + (p + 1) * 128],
                            rhs=xT[:, r * 128:(r + 1) * 128],
                            start=(r == 0),
                            stop=(r == 15),
                        )
                    if scl != 1.0:
                        nc.scalar.mul(stg[:, p * 128:(p + 1) * 128], ps[:, 0:128], scl)
                    else:
                        nc.scalar.copy(stg[:, p * 128:(p + 1) * 128], ps[:, 0:128])
            # re-base to [64, hl*128 + (b,s)] layout
            for hl in range(4):
                p, hf = hl // 2, hl % 2
                nc.gpsimd.dma_start(
                    out=qT2[:, hl * 128:(hl + 1) * 128],
                    in_=qstage[hf * 64:(hf + 1) * 64, p * 128:(p + 1) * 128],
                )
                nc.gpsimd.dma_start(
                    out=kTn[:, hl * 128:(hl + 1) * 128],
                    in_=kstage[hf * 64:(hf + 1) * 64, p * 128:(p + 1) * 128],
                )

            # v projection [128 tok, 256]
            psv = psB.tile([128, 256], F32, tag="sc")
            for r in range(16):
                nc.tensor.matmul(
                    psv[:, :],
                    lhsT=xT[:, r * 128:(r + 1) * 128],
                    rhs=wvT[:, r * 256:(r + 1) * 256],
                    start=(r == 0),
                    stop=(r == 15),
                )
            vn_sb = finp.tile([128, 256], BF16, tag="vnsb")
            nc.scalar.copy(vn_sb[:, :], psv[:, :])
            vn2v = vn2[:, :].rearrange("s (pr c) -> s pr c", pr=32, c=65)
            nc.vector.memset(vn2v[:, :, 64:65], 1.0)
            for b2 in range(8):
                nc.gpsimd.dma_start(
                    out=vn2[:, :].rearrange("s (hl b c) -> s hl b c",
                                            hl=4, b=8, c=65)[:, :, b2, 0:64],
                    in_=vn_sb[b2 * 16:(b2 + 1) * 16, :].rearrange(
                        "s (hl d) -> s hl d", hl=4
                    ),
                )

            # ---------- main attention loop ----------
            # (final out DMA emitted after the loop)
            if main_part < 4:
                nc.vector.memset(attnS[:, :], 0.0)
            for b in range(B):
                for hl in range(HLOC):
                    pidx = hl * 8 + b
                    jp = b * HLOC + hl
                    if jp in prefetch:
                        kb, vb = prefetch.pop(jp)
                    else:
                        kb, vb = load_kv(b, hl)
                    # v: cast + restride 64 -> 65 cols (ones in col 64)
                    vbf = vbfp.tile([128, 32 * 65], BF16, tag="vbf")
                    nc.vector.memset(
                        vbf[:, :].rearrange("p (t c) -> p t c", t=32, c=65)[:, :, 64:65],
                        1.0,
                    )
                    nc.vector.tensor_copy(
                        vbf[:, :].rearrange("p (t c) -> p t c", t=32, c=65)[:, :, 0:64],
                        vb[:, :].rearrange("p (t d) -> p t d", t=32, d=64),
                    )
                    # K tiles -> kT [64, 32*128] bf16, all at partitions 0:64
                    kt = ktp.tile([64, 32 * 128], BF16, tag="kt")
                    for gg in range(8):
                        ps = psA.tile([64, 512], F32, tag="trp")
                        for j in range(4):
                            t = gg * 4 + j
                            nc.tensor.transpose(
                                ps[:, j * 128:(j + 1) * 128],
                                kb[:, t * 64:(t + 1) * 64],
                                ident[:, :],
                            )
                        if gg % 2 == 1:
                            nc.vector.tensor_copy(kt[:, gg * 512:(gg + 1) * 512], ps[:, :])
                        else:
                            nc.scalar.copy(kt[:, gg * 512:(gg + 1) * 512], ps[:, :])
                    if main_part < 2:
                        continue
                    # scores^T + exp
                    pt = probsp.tile([128, 544], BF16, tag="pt")
                    qsl = qT2[:, hl * 128 + b * 16: hl * 128 + (b + 1) * 16]
                    knsl = kTn[:, hl * 128 + b * 16: hl * 128 + (b + 1) * 16]
                    for t16 in range(3):
                        if t16 < 2:
                            ps_sc = psB.tile([128, 256], F32, tag="sc")
                            for j in range(16):
                                t = t16 * 16 + j
                                nc.tensor.matmul(
                                    ps_sc[:, j * 16:(j + 1) * 16],
                                    lhsT=kt[:, t * 128:(t + 1) * 128],
                                    rhs=qsl,
                                    start=True,
                                    stop=True,
                                )
                            nc.scalar.activation(
                                pt[:, t16 * 256:(t16 + 1) * 256], ps_sc[:, :], EXP
                            )
                        else:
                            ps_sc = psB.tile([128, 256], F32, tag="sc")
                            nc.tensor.matmul(
                                ps_sc[0:16, 0:16],
                                lhsT=knsl,
                                rhs=qsl,
                                start=True,
                                stop=True,
                            )
                            nc.scalar.activation(pt[0:16, 512:528], ps_sc[0:16, 0:16], EXP)
                    if main_part < 3:
                        continue
                    # out2^T accumulation [65, 16]; row 64 = denom
                    po = psC.tile([65, 16], F32, tag="out2")
                    for t in range(32):
                        nc.tensor.matmul(
                            po[:, :],
                            lhsT=vbf[:, t * 65:(t + 1) * 65],
                            rhs=pt[:, t * 16:(t + 1) * 16],
                            start=(t == 0),
                            stop=False,
                        )
                    nc.tensor.matmul(
                        po[:, :],
                        lhsT=vn2[0:16, pidx * 65:(pidx + 1) * 65],
                        rhs=pt[0:16, 512:528],
                        start=False,
                        stop=True,
                    )
                    if main_part < 4:
                        continue
                    # finalize: normalize into attnS (base 0)
                    rec1 = finp.tile([1, 16], F32, tag="rec")
                    nc.vector.reciprocal(rec1[:, :], po[64:65, :])
                    recb = finp.tile([64, 16], F32, tag="recb")
                    nc.gpsimd.partition_broadcast(recb[:, :], rec1[:, :])
                    nc.vector.tensor_tensor(
                        attnS[:, hl * 128 + b * 16: hl * 128 + (b + 1) * 16],
                        po[0:64, :],
                        recb[:, :],
                        mybir.AluOpType.mult,
                    )

                # chunked (3/2/2/1 batches): assemble attnT, out-proj, AllReduce
                CH_END = {1: (0, 32), 3: (32, 64), 5: (64, 96), 7: (96, 128)}
                if b in CH_END and main_part >= 4:
                    c0, c1 = CH_END[b]
                    for hl2 in range(4):
                        p2, hf2 = hl2 // 2, hl2 % 2
                        nc.gpsimd.dma_start(
                            out=attnT[hf2 * 64:(hf2 + 1) * 64,
                                      p2 * 128 + c0: p2 * 128 + c1],
                            in_=attnS[:, hl2 * 128 + c0: hl2 * 128 + c1],
                        )
                    och = finp.tile([c1 - c0, 2048], F32, tag="och")
                    for n in range(4):
                        pso = psC.tile([c1 - c0, 512], F32, tag="out2")
                        for kc in range(2):
                            nc.tensor.matmul(
                                pso[:, :],
                                lhsT=attnT[:, kc * 128 + c0: kc * 128 + c1],
                                rhs=woT[:, kc * 2048 + n * 512: kc * 2048 + (n + 1) * 512],
                                start=(kc == 0),
                                stop=(kc == 1),
                            )
                        nc.vector.tensor_copy(och[:, n * 512:(n + 1) * 512], pso[:, :])
                    if skip_cc:
                        nc.gpsimd.dma_start(out=out_e[c0:c1, :], in_=och[:, :])
                    else:
                        nc.gpsimd.dma_start(out=cc_in[c0:c1, :], in_=och[:, :])
                        ch = c0 // 32
                        nc.gpsimd.collective_compute(
                            "ReduceScatter",
                            mybir.AluOpType.add,
                            replica_groups=[list(range(NCORES))],
                            ins=[cc_in[c0:c1, :].opt()],
                            outs=[cc_out[ch * 4:(ch + 1) * 4, :].opt()],
                        )
            if not skip_cc:
                nc.sync.dma_start(out=out_e[:, :], in_=cc_out[:, :])

    nc.compile()
    return nc


_CACHE = {}


def _get_nc():
    if "nc" not in _CACHE:
        _CACHE["nc"] = build_nc()
    return _CACHE["nc"]


def make_in_maps(hidden_states, past_k, past_v, wq, wk, wv, wo):
    x = np.ascontiguousarray(np.asarray(hidden_states, np.float32).reshape(NTOK, HID))
    wq = np.asarray(wq, np.float32)
    wk = np.asarray(wk, np.float32)
    wv = np.asarray(wv, np.float32)
    wo = np.asarray(wo, np.float32)
    past_k = np.asarray(past_k, np.float32)
    past_v = np.asarray(past_v, np.float32)
    in_maps = []
    for c in range(NCORES):
        rs = slice(c * SH, (c + 1) * SH)
        in_maps.append({
            "hidden": x,
            "wq": np.ascontiguousarray(wq[rs, :]),
            "wk": np.ascontiguousarray(wk[rs, :]),
            "wv": np.ascontiguousarray(wv[rs, :]),
            "wo": np.ascontiguousarray(wo[:, rs]),
            "pk": np.ascontiguousarray(past_k[:, c * HLOC:(c + 1) * HLOC]),
            "pv": np.ascontiguousarray(past_v[:, c * HLOC:(c + 1) * HLOC]),
        })
    return in_maps


def assemble_out(results):
    # each core's "out" is its ReduceScatter shard: rows [4c, 4c+4) of each
    # 32-row chunk's sum; stitch the full [128, 2048] from all cores
    out = np.empty((NTOK, HID), np.float32)
    for c in range(NCORES):
        shard = np.asarray(results[c]["out"], np.float32)
        for ch in range(4):
            out[ch * 32 + 4 * c: ch * 32 + 4 * c + 4] = shard[ch * 4:(ch + 1) * 4]
    return out


def kernel(hidden_states, past_k, past_v, wq, wk, wv, wo):
    nc = _get_nc()
    in_maps = make_in_maps(hidden_states, past_k, past_v, wq, wk, wv, wo)
    res = run_bass_kernel_spmd(nc, in_maps, core_ids=list(range(NCORES)))
    return assemble_out(res.results).reshape(B, S, HID)

